# revision 10
# baseline (speedup 1.0000x reference)
"""BRD4KANModel Trainium2 kernel.

Data-parallel over batch across 8 NeuronCores (512 rows each, weights
replicated). On-chip layout is feature-major (h^T: features on partitions,
batch on the free dim), so every layer's matmul output [out_feat, batch]
feeds the next layer directly. Weights arrive (out, in) row-major; the PE
needs the contraction dim on partitions, so each 128x128 weight tile is
cast to bf16 during the SWDGE DMA and transposed on-chip with a PE
transpose (is_transpose matmul vs identity), evacuated PSUM->SBUF by the
scalar engine.

B-spline bases use the truncated-power form: with z_m = lam*relu(x - g_m),
lam = (6h^3)^(-1/3), the 6 cubic bases are the 4th forward differences of
z_m^3 — identical to the Cox-de Boor reference (and exactly 0 outside the
grid) up to fp32 cancellation ~1e-3 absolute.

This walrus build accepts only ONE semaphore wait per instruction, while
Tile's scheduler attaches several; _split_waits() post-processes the BIR
JSON, hoisting excess waits onto NoOps inserted just before each
instruction on the same engine.
"""

import json
import os

import numpy as np

import concourse.bass as bass
import concourse.mybir as mybir
import concourse.tile as tile
from concourse.masks import make_identity

F32 = mybir.dt.float32
BF16 = mybir.dt.bfloat16
AF = mybir.ActivationFunctionType
OP = mybir.AluOpType

N_CORES = 8
BATCH = 4096
B = BATCH // N_CORES  # 512 per core
D = 2048
WIDTHS = [2048, 2048, 1024]
COEFF = 6
GRID_SIZE = 3
SPLINE_ORDER = 3
H = 2.0 / GRID_SIZE
GRID = [m * H - 1.0 - SPLINE_ORDER * H for m in range(GRID_SIZE + 2 * SPLINE_ORDER + 1)]
LAM = float((6.0 * H**3) ** (-1.0 / 3.0))

CH = 256          # spline i-chunk (features per weight DMA chunk)
ZW = 256          # bases compute width (sub-batch per DVE pass)


def _split_waits(bir_bytes: bytes, keep: int = 1) -> bytes:
    d = json.loads(bir_bytes)
    for f in d["functions"]:
        for bb in f["blocks"]:
            new_insts = []
            for inst in bb["instructions"]:
                si = inst.get("sync_info")
                waits = (si or {}).get("on_wait") or []
                if len(waits) > keep:
                    extra = waits[:-keep]
                    inst["sync_info"]["on_wait"] = waits[-keep:]
                    for ci in range(0, len(extra), keep):
                        new_insts.append({
                            "name": f"{inst['name']}-w{ci}",
                            "opcode": "NoOp",
                            "engine": inst["engine"],
                            "ins": [],
                            "outs": [],
                            "debug": inst.get("debug"),
                            "sync_info": {"on_update": [],
                                          "on_wait": extra[ci:ci + keep]},
                        })
                new_insts.append(inst)
            bb["instructions"] = new_insts
    return json.dumps(d).encode()


def _patch_json(nc):
    orig = nc.to_json_bytes

    def patched():
        return _split_waits(orig())

    nc.to_json_bytes = patched
    return nc


def build(stage=99):
    nc = bass.Bass()
    x = nc.dram_tensor("x", [B, D], F32, kind="ExternalInput")
    mult_w = nc.dram_tensor("mult_w", [2 * D, D], F32, kind="ExternalInput")
    mult_b = nc.dram_tensor("mult_b", [2 * D], F32, kind="ExternalInput")
    kan = []
    dims = [D] + WIDTHS
    for l in range(3):
        fo = dims[l + 1]
        kan.append((
            nc.dram_tensor(f"base_w{l}", [fo, dims[l]], F32, kind="ExternalInput"),
            nc.dram_tensor(f"spline_w{l}", [fo, dims[l], COEFF], F32, kind="ExternalInput"),
            nc.dram_tensor(f"scaler{l}", [fo, dims[l]], F32, kind="ExternalInput"),
        ))
    reg_w = nc.dram_tensor("reg_w", [1, WIDTHS[-1]], F32, kind="ExternalInput")
    reg_b = nc.dram_tensor("reg_b", [1], F32, kind="ExternalInput")
    aux_w = nc.dram_tensor("aux_w", [1, WIDTHS[-1]], F32, kind="ExternalInput")
    aux_b = nc.dram_tensor("aux_b", [1], F32, kind="ExternalInput")
    out = nc.dram_tensor("out", [2, B], F32, kind="ExternalOutput")
    dbg = nc.dram_tensor("dbg", [128, B], F32, kind="ExternalOutput")

    with tile.TileContext(nc) as tc:
        with tc.tile_pool(name="consts", bufs=1) as consts, \
             tc.tile_pool(name="hp", bufs=18) as hp, \
             tc.tile_pool(name="rhs", bufs=16) as rhsp, \
             tc.tile_pool(name="bases", bufs=96) as basesp, \
             tc.tile_pool(name="zp", bufs=11) as zp, \
             tc.tile_pool(name="z2p", bufs=2) as z2p, \
             tc.tile_pool(name="wload", bufs=2) as wload, \
             tc.tile_pool(name="scload", bufs=2) as scload, \
             tc.tile_pool(name="sp", bufs=2) as spp, \
             tc.tile_pool(name="sw", bufs=2) as swp, \
             tc.tile_pool(name="wT", bufs=8) as wTp, \
             tc.tile_pool(name="h2", bufs=8) as h2p, \
             tc.tile_pool(name="psA", bufs=4, space="PSUM") as psA, \
             tc.tile_pool(name="psT", bufs=4, space="PSUM") as psT:

            ident = consts.tile([128, 128], BF16, tag="ident")
            make_identity(nc, ident)
            mb_sb = consts.tile([128, 32], F32, tag="mb")
            nc.sync.dma_start(mb_sb, mult_b[:].rearrange("(t p) -> p t", p=128))
            hw_sb = consts.tile([2, 1024], BF16, tag="hw")
            nc.gpsimd.dma_start(hw_sb[0:1, :], reg_w[:])
            nc.gpsimd.dma_start(hw_sb[1:2, :], aux_w[:])
            hb_sb = consts.tile([2, 1], F32, tag="hb")
            nc.sync.dma_start(hb_sb[0:1, :], reg_b[None, :])
            nc.sync.dma_start(hb_sb[1:2, :], aux_b[None, :])
            grid_sb = consts.tile([128, 10], F32, tag="grid")
            for m in range(10):
                nc.vector.memset(grid_sb[:, m:m + 1], float(-LAM * GRID[m]))

            def transpose_tile(src_ap):
                """src [128,128] bf16 (any strided slice) -> transposed SBUF bf16."""
                pt = psT.tile([128, 128], BF16, tag="pt")
                nc.tensor.transpose(pt, src_ap, ident)
                dst = wTp.tile([128, 128], BF16, tag="wT")
                nc.scalar.copy(dst, pt)
                return dst

            # ---- x^T: cast x to bf16 and PE-transpose into feature-major ----
            IT0 = D // 128  # 16
            xb = []  # xb[i] [128, B] bf16, partitions = features
            for i in range(IT0):
                xb.append(rhsp.tile([128, B], BF16, tag="rhs", name=f"xb{i}"))
            for bt in range(B // 128):  # 4 batch tiles
                xf = wload.tile([128, D], BF16, tag="wload")
                nc.gpsimd.dma_start(xf, x[bt * 128:(bt + 1) * 128, :])
                for i in range(IT0):
                    pt = psT.tile([128, 128], BF16, tag="pt")
                    nc.tensor.transpose(pt, xf[:, i * 128:(i + 1) * 128], ident)
                    nc.scalar.copy(xb[i][:, bt * 128:(bt + 1) * 128], pt)

            # ---- multiplicative layer: hh = x @ mult_w.T + b; h=sig(gate)*val
            h_tiles = []
            for j in range(IT0):  # output tiles of h (2048 feats)
                sig = None
                for half, o in ((0, j), (1, j + 16)):
                    acc = psA.tile([128, B], F32, tag="acc")
                    wstrip = wload.tile([128, D], BF16, tag="wload")
                    nc.gpsimd.dma_start(wstrip, mult_w[o * 128:(o + 1) * 128, :])
                    for i in range(IT0):
                        wT = transpose_tile(wstrip[:, i * 128:(i + 1) * 128])
                        nc.tensor.matmul(acc, wT, xb[i],
                                         start=(i == 0), stop=(i == IT0 - 1))
                    if half == 0:
                        sig = hp.tile([128, B], F32, tag="h")
                        nc.scalar.activation(sig, acc, AF.Sigmoid,
                                             bias=mb_sb[:, j:j + 1])
                    else:
                        val = hp.tile([128, B], F32, tag="h")
                        nc.vector.tensor_scalar(val, acc, mb_sb[:, 16 + j:17 + j],
                                                None, OP.add)
                        ht = hp.tile([128, B], F32, tag="h")
                        nc.vector.tensor_tensor(ht, sig, val, OP.mult)
                        h_tiles.append(ht)

            # ---- KAN layers ----
            for l in range(min(3, max(0, stage - 1))):
                bw_d, sw_d, sc_d = kan[l]
                fi, fo = dims[l], dims[l + 1]
                IT, OT = fi // 128, fo // 128
                last = (l == 2)

                # phase A: silu + bases from h_tiles
                silu = []
                bas = []  # bas[i][c]
                for i in range(IT):
                    st = rhsp.tile([128, B], BF16, tag="rhs")
                    nc.scalar.activation(st, h_tiles[i], AF.Silu)
                    silu.append(st)
                    bt6 = [basesp.tile([128, B], BF16, tag="bases", name=f"bas{l}_{i}_{c}")
                           for c in range(COEFF)]
                    bas.append(bt6)
                    for w0 in range(0, B, ZW):
                        sl = slice(w0, w0 + ZW)
                        z = []
                        z2 = z2p.tile([128, ZW], F32, tag="z2")
                        for m in range(10):
                            zm = zp.tile([128, ZW], F32, tag="z", name=f"z{m}")
                            nc.scalar.activation(zm, h_tiles[i][:, sl], AF.Relu,
                                                 bias=grid_sb[:, m:m + 1],
                                                 scale=LAM)
                            nc.vector.tensor_tensor(z2, zm, zm, OP.mult)
                            nc.vector.tensor_tensor(zm, z2, zm, OP.mult)
                            z.append(zm)
                        for r in range(3):
                            for m in range(9 - r):
                                nc.vector.tensor_tensor(z[m], z[m], z[m + 1],
                                                        OP.subtract)
                        for c in range(COEFF):
                            nc.vector.tensor_tensor(bt6[c][:, sl], z[c], z[c + 1],
                                                    OP.subtract)

                # phase B: weights, transposes, matmuls
                new_h = []
                K_TOT = IT * 7
                for o in range(OT):
                    acc = psA.tile([128, B], F32, tag="acc")
                    osl = slice(o * 128, (o + 1) * 128)
                    bstrip = wload.tile([128, fi], BF16, tag="wload")
                    nc.gpsimd.dma_start(bstrip, bw_d[osl, :])
                    k = 0
                    for i in range(IT):
                        wT = transpose_tile(bstrip[:, i * 128:(i + 1) * 128])
                        nc.tensor.matmul(acc, wT, silu[i],
                                         start=(k == 0), stop=(k == K_TOT - 1))
                        k += 1
                    for ic in range(fi // CH):
                        spt = spp.tile([128, CH * COEFF], BF16, tag="sp")
                        nc.gpsimd.dma_start(
                            spt, sw_d[osl, ic * CH:(ic + 1) * CH, :]
                            .rearrange("o i c -> o (i c)"))
                        sct = scload.tile([128, CH], BF16, tag="sc")
                        nc.gpsimd.dma_start(sct, sc_d[osl, ic * CH:(ic + 1) * CH])
                        swt = swp.tile([128, CH * COEFF], BF16, tag="sw")
                        sp3 = spt.rearrange("p (i c) -> p i c", c=COEFF)
                        sw3 = swt.rearrange("p (i c) -> p i c", c=COEFF)
                        nc.vector.tensor_tensor(
                            sw3, sp3,
                            sct[:, :, None].to_broadcast(sp3.shape), OP.mult)
                        for isub in range(CH // 128):
                            i_g = ic * (CH // 128) + isub
                            for c in range(COEFF):
                                view = sw3[:, isub * 128:(isub + 1) * 128, c]
                                wT = transpose_tile(view)
                                nc.tensor.matmul(acc, wT, bas[i_g][c],
                                                 start=(k == 0),
                                                 stop=(k == K_TOT - 1))
                                k += 1
                    if last:
                        ot = h2p.tile([128, B], BF16, tag="h2")
                    else:
                        ot = hp.tile([128, B], F32, tag="h")
                    nc.scalar.copy(ot, acc)
                    new_h.append(ot)
                h_tiles = new_h

            # ---- debug tap: first live tile of h_tiles ----
            if stage < 5:
                dbg_t = hp.tile([128, B], F32, tag="h", name="dbgt")
                nc.vector.tensor_copy(dbg_t, h_tiles[0])
                nc.sync.dma_start(dbg[:], dbg_t)

            # ---- heads ----
            if stage >= 5:
                acc = psA.tile([128, B], F32, tag="acc")
                IT2 = WIDTHS[-1] // 128  # 8
                for i in range(IT2):
                    pt = psT.tile([128, 128], BF16, tag="pt")
                    nc.tensor.transpose(pt[:, 0:2],
                                        hw_sb[:, i * 128:(i + 1) * 128],
                                        ident[0:2, 0:2])
                    wT = wTp.tile([128, 128], BF16, tag="wT")
                    nc.scalar.copy(wT[:, 0:2], pt[:, 0:2])
                    nc.tensor.matmul(acc[0:2, :], wT[:, 0:2], h_tiles[i],
                                     start=(i == 0), stop=(i == IT2 - 1))
                res = consts.tile([2, B], F32, tag="res")
                nc.vector.tensor_scalar(res, acc[0:2, :], hb_sb[:, 0:1], None,
                                        OP.add)
                nc.sync.dma_start(out[:], res)

    return _patch_json(nc)


_NC = None


def kernel(**inputs):
    global _NC
    from concourse.bass_utils import run_bass_kernel_spmd

    if _NC is None:
        _NC = build(int(os.environ.get("KSTAGE", "99")))
    per_core = []
    x_full = np.ascontiguousarray(inputs["x"], dtype=np.float32)
    shared = {k: np.ascontiguousarray(np.asarray(v), dtype=np.float32)
              for k, v in inputs.items() if k != "x"}
    for c in range(N_CORES):
        m = dict(shared)
        m["x"] = np.ascontiguousarray(x_full[c * B:(c + 1) * B])
        per_core.append(m)
    res = run_bass_kernel_spmd(_NC, per_core, core_ids=list(range(N_CORES)))
    reg = np.concatenate([res.results[c]["out"][0] for c in range(N_CORES)])
    aux = np.concatenate([res.results[c]["out"][1] for c in range(N_CORES)])
    kernel.last_results = res
    return reg, aux
